# revision 20
# baseline (speedup 1.0000x reference)
import math
import os

import ml_dtypes
import numpy as np

BF16 = ml_dtypes.bfloat16


class Cfg:
    def __init__(self, n_nodes=50000, n_cores=8, f_in=128, f_hid=32, heads=4,
                 n_class=40, alpha=0.2):
        self.N = n_nodes
        self.C = n_cores
        self.NSH = n_nodes // n_cores
        self.WIN = 128
        self.NW = math.ceil(self.NSH / 128)
        self.NSHP = self.NW * 128
        self.HALFROWS = self.C * self.NSHP // 2
        self.F = f_in
        self.FH = f_hid
        self.H = heads
        self.NC2 = n_class
        self.ALPHA = alpha
        self.GRP = 3
        self.CHK = 6


CFG = Cfg()



def plan(cfg, src, tgt):
    C, NSH, NW = cfg.C, cfg.NSH, cfg.NW
    core = tgt // NSH
    tl = tgt - core * NSH
    win = tl // 128
    row = core * cfg.NSHP + tl
    srow = (src // NSH) * cfg.NSHP + (src % NSH)
    hf = (srow >= cfg.HALFROWS).astype(np.int64)

    counts = np.zeros((C, NW, 2), np.int64)
    np.add.at(counts, (core, win, hf), 1)
    ntiles = np.ceil(counts.max(axis=0) / 128).astype(np.int64)

    meta = []
    t0 = 0
    for w in range(NW):
        chunks = []
        for h in range(2):
            nt = int(ntiles[w, h])
            t = 0
            while t < nt:
                nc_ = min(cfg.CHK, nt - t)
                chunks.append((h, t0 + t, nc_))
                t += nc_
            t0 += nt
        meta.append(chunks)
    Ttot = t0

    base = np.zeros((NW, 2), np.int64)
    acc = 0
    for w in range(NW):
        for h in range(2):
            base[w, h] = acc
            acc += int(ntiles[w, h])

    order = np.lexsort((src, hf, win, core))
    percore = []
    for c in range(C):
        lo = np.searchsorted(core[order], c)
        hi = np.searchsorted(core[order], c + 1)
        oc = order[lo:hi]
        wc, hc = win[oc], hf[oc]
        keys = wc * 2 + hc
        grp_start = np.searchsorted(keys, np.arange(NW * 2))
        pos = np.arange(len(oc)) - grp_start[keys]
        slots = (base[wc, hc] * 128 + pos).astype(np.int64)
        percore.append(dict(e=oc, slots=slots))
    return meta, Ttot, percore


def build_inputs(cfg, x, edge_index, W1, a1_src, a1_tgt, b1, W2, a2_src,
                 a2_tgt, meta, Ttot, percore):
    src = np.asarray(edge_index[0], np.int64)
    tgt = np.asarray(edge_index[1], np.int64)
    srow = (src // cfg.NSH) * cfg.NSHP + (src % cfg.NSH)
    xf = np.asarray(x, np.float32)
    NSLOT = Ttot * 128

    iota = np.tile(np.arange(128, dtype=np.float32), (128, 1)).astype(BF16)
    mhead = np.zeros((cfg.H, 128), np.float32)
    for h in range(cfg.H):
        mhead[h, h * cfg.FH:(h + 1) * cfg.FH] = 1.0
    ident = np.eye(128, dtype=np.float32)

    common = {
        "W1": np.asarray(W1, np.float32),
        "A1S": np.asarray(a1_src, np.float32).reshape(1, -1),
        "A1T": np.asarray(a1_tgt, np.float32).reshape(1, -1),
        "B1": np.asarray(b1, np.float32).reshape(cfg.F, 1),
        "W2": np.asarray(W2, np.float32),
        "A2S": np.asarray(a2_src, np.float32).reshape(1, cfg.NC2),
        "A2T": np.asarray(a2_tgt, np.float32).reshape(1, cfg.NC2),
        "IOTA": iota,
        "MHEAD": mhead,
        "IDENT": ident,
        "IDENTB": ident.astype(BF16),
    }

    in_maps = []
    for c in range(cfg.C):
        pc = percore[c]
        e, slots = pc["e"], pc["slots"]
        xs = np.zeros((NSLOT, cfg.F), np.float32)
        xs[slots] = xf[src[e]]
        xt = np.zeros((NSLOT, cfg.F), np.float32)
        xt[slots] = xf[tgt[e]]
        wr = np.full(NSLOT, -1.0, np.float32)
        wr[slots] = (tgt[e] % cfg.NSH) % 128
        idxf = np.zeros(NSLOT, np.int64)
        idxf[slots] = srow[e] % cfg.HALFROWS
        l2i = np.zeros((128, Ttot * 8), np.int16)
        for chunks in meta:
            for (h, t0, nt) in chunks:
                ni = nt * 128
                v = idxf[t0 * 128:(t0 + nt) * 128].astype(np.int16)
                l2i[:, t0 * 8:(t0 + nt) * 8] = np.tile(
                    v.reshape(ni // 16, 16).T, (8, 1))
        m = dict(common)
        m["XS"] = np.ascontiguousarray(xs.T.astype(BF16))
        m["XT"] = np.ascontiguousarray(xt.T.astype(BF16))
        m["WR"] = np.ascontiguousarray(
            wr.reshape(Ttot, 128).T.astype(BF16))
        m["L2I"] = l2i
        in_maps.append(m)
    return in_maps



def build_kernel(cfg, meta, Ttot):
    import concourse.bacc as bacc
    import concourse.bass as bass
    import concourse.tile as tile
    import concourse.mybir as mybir

    dt = mybir.dt
    AF = mybir.ActivationFunctionType
    OP = mybir.AluOpType

    def bc(ap, n):
        return bass.AP(ap.tensor, ap.offset, list(ap.ap) + [[0, n]])

    def ins(ap, pos, n):
        l = list(ap.ap)
        return bass.AP(ap.tensor, ap.offset, l[:pos] + [[0, n]] + l[pos:])

    nc = bacc.Bacc("TRN2", target_bir_lowering=False, debug=False,
                   num_devices=cfg.C)
    F, H, FH, NC2 = cfg.F, cfg.H, cfg.FH, cfg.NC2
    NW, NSHP = cfg.NW, cfg.NSHP

    XS = nc.dram_tensor("XS", [128, Ttot * 128], dt.bfloat16, kind="ExternalInput")
    XT = nc.dram_tensor("XT", [128, Ttot * 128], dt.bfloat16, kind="ExternalInput")
    WR = nc.dram_tensor("WR", [128, Ttot], dt.bfloat16, kind="ExternalInput")
    L2I = nc.dram_tensor("L2I", [128, Ttot * 8], dt.int16, kind="ExternalInput")
    W1 = nc.dram_tensor("W1", [H, F, FH], dt.float32, kind="ExternalInput")
    A1S = nc.dram_tensor("A1S", [1, H * FH], dt.float32, kind="ExternalInput")
    A1T = nc.dram_tensor("A1T", [1, H * FH], dt.float32, kind="ExternalInput")
    B1 = nc.dram_tensor("B1", [F, 1], dt.float32, kind="ExternalInput")
    W2 = nc.dram_tensor("W2", [F, NC2], dt.float32, kind="ExternalInput")
    A2S = nc.dram_tensor("A2S", [1, NC2], dt.float32, kind="ExternalInput")
    A2T = nc.dram_tensor("A2T", [1, NC2], dt.float32, kind="ExternalInput")
    IOTA = nc.dram_tensor("IOTA", [128, 128], dt.bfloat16, kind="ExternalInput")
    MHEAD = nc.dram_tensor("MHEAD", [H, 128], dt.float32, kind="ExternalInput")
    IDENT = nc.dram_tensor("IDENT", [128, 128], dt.float32, kind="ExternalInput")
    IDENTB = nc.dram_tensor("IDENTB", [128, 128], dt.bfloat16, kind="ExternalInput")
    OUT = nc.dram_tensor("OUT", [cfg.NSH, NC2], dt.float32, kind="ExternalOutput")
    DBG = None
    if getattr(cfg, "debug", False):
        DBG = nc.dram_tensor("DBG", [128, cfg.NW * 128], dt.float32,
                             kind="ExternalOutput")

    with tile.TileContext(nc) as tc:
        with (
            tc.tile_pool(name="const", bufs=1) as cons,
            tc.tile_pool(name="dram", bufs=1, space="DRAM") as dram,
            nc.semaphore("gsem") as gsem,
        ):
            agin = dram.tile([NSHP, 128], dt.bfloat16)
            agout = dram.tile([cfg.C * NSHP, 128], dt.bfloat16)

            w1f = cons.tile([128, H, FH], dt.float32)
            nc.sync.dma_start(w1f[:], W1.ap().rearrange("h f o -> f h o"))
            w1bf = cons.tile([128, 128], dt.bfloat16)
            nc.vector.tensor_copy(w1bf[:], w1f[:].rearrange("f h o -> f (h o)"))
            ones1 = cons.tile([1, 128], dt.float32)
            nc.gpsimd.memset(ones1[:], 1.0)
            iota_sb = cons.tile([128, 128], dt.bfloat16)
            nc.sync.dma_start(iota_sb[:], IOTA.ap())
            mhead_sb = cons.tile([H, 128], dt.float32)
            nc.sync.dma_start(mhead_sb[:], MHEAD.ap())
            identb = cons.tile([128, 128], dt.bfloat16)
            nc.sync.dma_start(identb[:], IDENTB.ap())
            identf = cons.tile([128, 128], dt.float32)
            nc.sync.dma_start(identf[:], IDENT.ap())
            b1sb = cons.tile([128, 1], dt.float32)
            nc.sync.dma_start(b1sb[:], B1.ap())

            with (
                tc.tile_pool(name="wprep", bufs=2) as wp,
                tc.tile_pool(name="wpsum", bufs=2, space="PSUM") as wps,
            ):
                a1_bf = []
                for nm, AT in (("s", A1S), ("t", A1T)):
                    aflat = wp.tile([1, 128], dt.float32)
                    nc.sync.dma_start(aflat[:], AT.ap())
                    pb = wps.tile([128, 128], dt.float32, tag="pb")
                    nc.tensor.matmul(pb[:], lhsT=ones1[:], rhs=aflat[:],
                                     start=True, stop=True)
                    prod = wp.tile([128, H, FH], dt.float32, tag="prod")
                    nc.vector.tensor_tensor(
                        out=prod[:], in0=w1f[:],
                        in1=pb[:].rearrange("f (h o) -> f h o", o=FH),
                        op=OP.mult)
                    red = cons.tile([128, H], dt.float32, tag=f"w1a1{nm}f")
                    nc.vector.reduce_sum(out=red[:], in_=prod[:],
                                         axis=mybir.AxisListType.X)
                    rbf = cons.tile([128, H], dt.bfloat16, tag=f"w1a1{nm}")
                    nc.vector.tensor_copy(rbf[:], red[:])
                    a1_bf.append(rbf)
                w1a1s, w1a1t = a1_bf

                w2f = wp.tile([128, NC2], dt.float32, tag="w2f")
                nc.sync.dma_start(w2f[:], W2.ap())
                rhs2 = cons.tile([128, NC2 + 1], dt.bfloat16)
                nc.vector.tensor_copy(rhs2[:, 0:NC2], w2f[:])
                w2a2t = cons.tile([128, 1], dt.bfloat16)
                for nm, AT in (("s", A2S), ("t", A2T)):
                    aflat = wp.tile([1, NC2], dt.float32, tag="a2flat")
                    nc.sync.dma_start(aflat[:], AT.ap())
                    pb = wps.tile([128, NC2], dt.float32, tag="pb2")
                    nc.tensor.matmul(pb[:], lhsT=ones1[:, 0:128], rhs=aflat[:],
                                     start=True, stop=True)
                    prod = wp.tile([128, NC2], dt.float32, tag="prod2")
                    nc.vector.tensor_tensor(out=prod[:], in0=w2f[:], in1=pb[:],
                                            op=OP.mult)
                    red = wp.tile([128, 1], dt.float32, tag="red2")
                    nc.vector.reduce_sum(out=red[:], in_=prod[:],
                                         axis=mybir.AxisListType.X)
                    if nm == "s":
                        nc.vector.tensor_copy(rhs2[:, NC2:NC2 + 1], red[:])
                    else:
                        nc.vector.tensor_copy(w2a2t[:], red[:])

            wr_sb = cons.tile([128, Ttot], dt.bfloat16)
            nc.sync.dma_start(wr_sb[:], WR.ap())
            hT = cons.tile([128, NSHP], dt.bfloat16)

            with (
                tc.tile_pool(name="xs", bufs=3) as xsp,
                tc.tile_pool(name="xt", bufs=3) as xtp,
                tc.tile_pool(name="oh", bufs=2) as ohp,
                tc.tile_pool(name="sc", bufs=2) as scp,
                tc.tile_pool(name="msg", bufs=2) as msgp,
                tc.tile_pool(name="epi", bufs=2) as epp,
                tc.tile_pool(name="pj", bufs=2, space="PSUM") as PJ,
                tc.tile_pool(name="ph", bufs=2, space="PSUM") as PH,
                tc.tile_pool(name="pd", bufs=2, space="PSUM") as PD,
                tc.tile_pool(name="pep", bufs=1, space="PSUM") as PEP,
            ):
                for w in range(NW):
                    chunks = meta[w]
                    tiles = []
                    for (hf, t0, ntc) in chunks:
                        t = 0
                        while t < ntc:
                            g = min(cfg.GRP, ntc - t)
                            tiles.append((t0 + t, g))
                            t += g
                    ph = PH.tile([128, 128], dt.float32)
                    pd = PD.tile([H, 128], dt.float32)
                    nmm = sum(g for (_, g) in tiles)
                    mmi = 0
                    for (t0, g) in tiles:
                        xs = xsp.tile([128, cfg.GRP * 128], dt.bfloat16)
                        nc.sync.dma_start(xs[:, 0:g * 128],
                                          XS.ap()[:, t0 * 128:(t0 + g) * 128])
                        xt = xtp.tile([128, cfg.GRP * 128], dt.bfloat16)
                        nc.sync.dma_start(xt[:, 0:g * 128],
                                          XT.ap()[:, t0 * 128:(t0 + g) * 128])
                        pj = PJ.tile([128, cfg.GRP, 136], dt.float32)
                        for j in range(g):
                            nc.tensor.matmul(
                                pj[:, j, 0:128],
                                lhsT=xs[:, j * 128:(j + 1) * 128],
                                rhs=w1bf[:], start=True, stop=True)
                            nc.tensor.matmul(
                                pj[:, j, 128:132],
                                lhsT=xs[:, j * 128:(j + 1) * 128],
                                rhs=w1a1s[:], start=True, stop=True)
                            nc.tensor.matmul(
                                pj[:, j, 132:136],
                                lhsT=xt[:, j * 128:(j + 1) * 128],
                                rhs=w1a1t[:], start=True, stop=True)
                        e1 = scp.tile([128, cfg.GRP, H], dt.float32, tag="e1")
                        pjb = pj[:]
                        pair = bass.AP(pjb.tensor, pjb.offset + 128,
                                       [list(pjb.ap[0]), [136, g], [1, H], [4, 2]])
                        nc.vector.tensor_reduce(out=e1[:, 0:g, :], in_=pair,
                                                op=OP.add,
                                                axis=mybir.AxisListType.X)
                        lr = scp.tile([128, cfg.GRP, H], dt.float32, tag="lr")
                        nc.vector.tensor_scalar(out=lr[:, 0:g, :],
                                                in0=e1[:, 0:g, :],
                                                scalar1=cfg.ALPHA, scalar2=None,
                                                op0=OP.mult)
                        nc.vector.tensor_tensor(out=lr[:, 0:g, :],
                                                in0=e1[:, 0:g, :],
                                                in1=lr[:, 0:g, :], op=OP.max)
                        exf = scp.tile([128, cfg.GRP, H], dt.float32, tag="exf")
                        nc.scalar.activation(exf[:, 0:g, :], lr[:, 0:g, :], AF.Exp)
                        exb = scp.tile([128, cfg.GRP, H], dt.bfloat16, tag="exb")
                        nc.gpsimd.tensor_copy(exb[:, 0:g, :], exf[:, 0:g, :])
                        oh = ohp.tile([128, cfg.GRP, 128], dt.bfloat16)
                        nc.vector.tensor_tensor(
                            out=oh[:, 0:g, :],
                            in0=bc(wr_sb[:, t0:t0 + g], 128),
                            in1=ins(iota_sb[:], 1, g),
                            op=OP.is_equal)
                        msg = msgp.tile([128, cfg.GRP, 128], dt.bfloat16)
                        nc.vector.tensor_tensor(
                            out=msg[:, 0:g, :].rearrange(
                                "p t (h o) -> p t h o", o=FH),
                            in0=pj[:, 0:g, 0:128].rearrange(
                                "p t (h o) -> p t h o", o=FH),
                            in1=bc(exf[:, 0:g, :], FH),
                            op=OP.mult)
                        for j in range(g):
                            nc.tensor.matmul(ph[:],
                                             lhsT=msg[:, j, :], rhs=oh[:, j, :],
                                             start=(mmi == 0),
                                             stop=(mmi == nmm - 1))
                            nc.tensor.matmul(pd[:],
                                             lhsT=exb[:, j, :], rhs=oh[:, j, :],
                                             start=(mmi == 0),
                                             stop=(mmi == nmm - 1))
                            mmi += 1
                    rec = epp.tile([H, 128], dt.float32, tag="rec")
                    nc.vector.tensor_scalar(out=rec[:], in0=pd[:],
                                            scalar1=1e-16, scalar2=None,
                                            op0=OP.add)
                    nc.vector.reciprocal(rec[:], rec[:])
                    pex = PEP.tile([128, 128], dt.float32)
                    nc.tensor.matmul(pex[:], lhsT=mhead_sb[:], rhs=rec[:],
                                     start=True, stop=True)
                    pexs = epp.tile([128, 128], dt.float32, tag="pexs")
                    nc.scalar.copy(pexs[:], pex[:])
                    y = epp.tile([128, 128], dt.float32, tag="y")
                    nc.vector.tensor_tensor(out=y[:], in0=ph[:],
                                            in1=pexs[:], op=OP.mult)
                    mn = epp.tile([128, 128], dt.float32, tag="mn")
                    nc.vector.tensor_scalar(out=mn[:], in0=y[:], scalar1=b1sb[:],
                                            scalar2=0.0, op0=OP.add, op1=OP.min)
                    mx = epp.tile([128, 128], dt.float32, tag="mx")
                    nc.vector.tensor_scalar(out=mx[:], in0=y[:], scalar1=b1sb[:],
                                            scalar2=0.0, op0=OP.add, op1=OP.max)
                    q = epp.tile([128, 128], dt.float32, tag="q")
                    nc.scalar.activation(q[:], mn[:], AF.Exp)
                    s_ = epp.tile([128, 128], dt.float32, tag="s_")
                    nc.vector.tensor_tensor(out=s_[:], in0=mx[:], in1=q[:],
                                            op=OP.add)
                    nc.vector.tensor_scalar(out=hT[:, w * 128:(w + 1) * 128],
                                            in0=s_[:], scalar1=-1.0, scalar2=None,
                                            op0=OP.add)
                    pht = PEP.tile([128, 128], dt.bfloat16, tag="pht")
                    nc.tensor.transpose(pht[:], hT[:, w * 128:(w + 1) * 128],
                                        identb[:])
                    hrow = epp.tile([128, 128], dt.bfloat16, tag="hrow")
                    nc.vector.tensor_copy(hrow[:], pht[:])
                    nc.sync.dma_start(agin[w * 128:(w + 1) * 128, :], hrow[:])

            if DBG is not None:
                nc.gpsimd.dma_start(DBG.ap(), hT[:])
            nc.gpsimd.collective_compute(
                "AllGather", OP.bypass,
                replica_groups=[list(range(cfg.C))],
                ins=[agin[:].opt()], outs=[agout[:].opt()])

            l2i_sb = cons.tile([128, Ttot * 8], dt.int16)
            nc.sync.dma_start(l2i_sb[:], L2I.ap())
            st2 = cons.tile([128, NW], dt.bfloat16)
            with tc.tile_pool(name="pn", bufs=2, space="PSUM") as PN:
                for w in range(NW):
                    pn = PN.tile([128, 1], dt.float32)
                    nc.tensor.matmul(pn[:], lhsT=hT[:, w * 128:(w + 1) * 128],
                                     rhs=w2a2t[:], start=True, stop=True)
                    nc.vector.tensor_copy(st2[:, w:w + 1], pn[:])

            gcount = 0
            with (
                tc.tile_pool(name="ge", bufs=3) as gep,
                tc.tile_pool(name="oh2", bufs=2) as ohp2,
                tc.tile_pool(name="oht", bufs=2) as ohtp,
                tc.tile_pool(name="sc2", bufs=2) as scp2,
                tc.tile_pool(name="m2", bufs=2) as m2p,
                tc.tile_pool(name="ep2", bufs=2) as ep2,
                tc.tile_pool(name="pj2", bufs=2, space="PSUM") as PJ2,
                tc.tile_pool(name="pot", bufs=2, space="PSUM") as POT,
                tc.tile_pool(name="po2", bufs=2, space="PSUM") as PO2,
                tc.tile_pool(name="pe2", bufs=1, space="PSUM") as PE2,
            ):
                for w in range(NW):
                    po2 = PO2.tile([NC2 + 1, 128], dt.float32)
                    tiles = []
                    for ci, (hf, t0, ntc) in enumerate(meta[w]):
                        ge = gep.tile([128, cfg.CHK * 128], dt.bfloat16)
                        ni = ntc * 128
                        gcount += 1
                        with tc.tile_critical():
                            nc.gpsimd.dma_gather(
                                ge[:, 0:ni].rearrange("p (one e) -> p one e",
                                                      one=1),
                                agout[hf * cfg.HALFROWS:
                                      (hf + 1) * cfg.HALFROWS, :],
                                l2i_sb[:, t0 * 8:(t0 + ntc) * 8],
                                ni, ni, 128, transpose=True,
                            ).then_inc(gsem, 16)
                            nc.gpsimd.wait_ge(gsem, 16 * gcount)
                        tiles.append((t0, ntc, ge))
                    nmm = sum(nt for (_, nt, _) in tiles)
                    mmi = 0
                    for (t0, ntc, ge) in tiles:
                        t = 0
                        while t < ntc:
                            g = min(cfg.GRP, ntc - t)
                            tg = t0 + t
                            pj2 = PJ2.tile([128, cfg.GRP, 42], dt.float32)
                            oh = ohp2.tile([128, cfg.GRP, 128], dt.bfloat16)
                            nc.vector.tensor_tensor(
                                out=oh[:, 0:g, :],
                                in0=bc(wr_sb[:, tg:tg + g], 128),
                                in1=ins(iota_sb[:], 1, g),
                                op=OP.is_equal)
                            pot = POT.tile([128, cfg.GRP, 128], dt.bfloat16)
                            for j in range(g):
                                nc.tensor.transpose(pot[:, j, :], oh[:, j, :],
                                                    identb[:])
                            oht = ohtp.tile([128, cfg.GRP, 128], dt.bfloat16)
                            nc.vector.tensor_copy(oht[:, 0:g, :], pot[:, 0:g, :])
                            for j in range(g):
                                eoff = (tg - t0 + j) * 128
                                nc.tensor.matmul(
                                    pj2[:, j, 0:41],
                                    lhsT=ge[:, eoff:eoff + 128],
                                    rhs=rhs2[:], start=True, stop=True)
                                nc.tensor.matmul(
                                    pj2[:, j, 41:42],
                                    lhsT=oht[:, j, :],
                                    rhs=st2[:, w:w + 1], start=True, stop=True)
                            e2 = scp2.tile([128, cfg.GRP, 1], dt.float32,
                                           tag="e2")
                            pj2b = pj2[:]
                            pair = bass.AP(pj2b.tensor, pj2b.offset + 40,
                                           [list(pj2b.ap[0]), [42, g], [1, 2]])
                            nc.vector.tensor_reduce(
                                out=e2[:, 0:g, :].rearrange(
                                    "p t one -> p (t one)"),
                                in_=pair, op=OP.add,
                                axis=mybir.AxisListType.X)
                            lr2 = scp2.tile([128, cfg.GRP, 1], dt.float32,
                                            tag="lr2")
                            nc.vector.tensor_scalar(out=lr2[:, 0:g, :],
                                                    in0=e2[:, 0:g, :],
                                                    scalar1=cfg.ALPHA,
                                                    scalar2=None, op0=OP.mult)
                            nc.vector.tensor_tensor(out=lr2[:, 0:g, :],
                                                    in0=e2[:, 0:g, :],
                                                    in1=lr2[:, 0:g, :],
                                                    op=OP.max)
                            ex2 = scp2.tile([128, cfg.GRP, 1], dt.float32,
                                            tag="ex2")
                            nc.scalar.activation(ex2[:, 0:g, :], lr2[:, 0:g, :],
                                                 AF.Exp)
                            msg2 = m2p.tile([128, cfg.GRP, NC2 + 1],
                                            dt.bfloat16)
                            nc.vector.tensor_tensor(
                                out=msg2[:, 0:g, 0:NC2],
                                in0=pj2[:, 0:g, 0:NC2],
                                in1=bc(ex2[:, 0:g, 0:1].rearrange(
                                    "p t one -> p (t one)"), NC2),
                                op=OP.mult)
                            nc.vector.tensor_copy(msg2[:, 0:g, NC2:NC2 + 1],
                                                  ex2[:, 0:g, :])
                            for j in range(g):
                                nc.tensor.matmul(po2[:],
                                                 lhsT=msg2[:, j, :],
                                                 rhs=oh[:, j, :],
                                                 start=(mmi == 0),
                                                 stop=(mmi == nmm - 1))
                                mmi += 1
                            t += g
                    o2c = ep2.tile([NC2 + 1, 128], dt.float32, tag="o2c")
                    nc.vector.tensor_copy(o2c[:], po2[:])
                    pt2 = PE2.tile([128, NC2 + 1], dt.float32)
                    nc.tensor.transpose(pt2[:], o2c[:],
                                        identf[0:NC2 + 1, 0:NC2 + 1])
                    rec2 = ep2.tile([128, 1], dt.float32, tag="rec2")
                    nc.vector.tensor_scalar(out=rec2[:], in0=pt2[:, NC2:NC2 + 1],
                                            scalar1=1e-16, scalar2=None,
                                            op0=OP.add)
                    nc.vector.reciprocal(rec2[:], rec2[:])
                    logit = ep2.tile([128, NC2], dt.float32, tag="logit")
                    nc.vector.tensor_scalar(out=logit[:], in0=pt2[:, 0:NC2],
                                            scalar1=rec2[:], scalar2=None,
                                            op0=OP.mult)
                    mx2 = ep2.tile([128, 1], dt.float32, tag="mx2")
                    nc.vector.reduce_max(out=mx2[:], in_=logit[:],
                                         axis=mybir.AxisListType.X)
                    z = ep2.tile([128, NC2], dt.float32, tag="z")
                    nc.vector.tensor_scalar(out=z[:], in0=logit[:],
                                            scalar1=mx2[:], scalar2=None,
                                            op0=OP.subtract)
                    ez = ep2.tile([128, NC2], dt.float32, tag="ez")
                    ssum = ep2.tile([128, 1], dt.float32, tag="ssum")
                    nc.scalar.activation(ez[:], z[:], AF.Exp, accum_out=ssum[:])
                    lg = ep2.tile([128, 1], dt.float32, tag="lg")
                    nc.scalar.activation(lg[:], ssum[:], AF.Ln)
                    o = ep2.tile([128, NC2], dt.float32, tag="o")
                    nc.vector.tensor_scalar(out=o[:], in0=z[:], scalar1=lg[:],
                                            scalar2=None, op0=OP.subtract)
                    rows = min(128, cfg.NSH - w * 128)
                    nc.sync.dma_start(OUT.ap()[w * 128:w * 128 + rows, :],
                                      o[0:rows, :])
    nc.compile()
    return nc



def _run(x, adj, edge_index, W1, a1_src, a1_tgt, b1, W2, a2_src, a2_tgt,
         trace=False):
    from concourse.bass_utils import run_bass_kernel_spmd

    cfg = CFG
    src = np.asarray(edge_index[0], np.int64)
    tgt = np.asarray(edge_index[1], np.int64)
    meta, Ttot, percore = plan(cfg, src, tgt)
    in_maps = build_inputs(cfg, x, edge_index, W1, a1_src, a1_tgt, b1, W2,
                           a2_src, a2_tgt, meta, Ttot, percore)
    nc = build_kernel(cfg, meta, Ttot)
    res = run_bass_kernel_spmd(nc, in_maps, list(range(cfg.C)), trace=trace)
    out = np.concatenate([res.results[c]["OUT"] for c in range(cfg.C)], axis=0)
    return out.astype(np.float32), res.exec_time_ns


def kernel(x, adj, edge_index, W1, a1_src, a1_tgt, b1, W2, a2_src, a2_tgt):
    return _run(x, adj, edge_index, W1, a1_src, a1_tgt, b1, W2, a2_src,
                a2_tgt)[0]
